# revision 89
# baseline (speedup 1.0000x reference)
"""Trainium2 Bass kernel for nn_AttentionBlock (B=4, H=W=64, C=64, GROUPS=32).

Math (reference):
    hn = GroupNorm(x; gamma, beta, 32 groups, eps=1e-3)
    q = hn@wq+bq ; k = hn@wk+bk ; v = hn@wv+bv
    att = softmax(q k^T / 8) over the 4096 spatial positions
    out = x + (att @ v) @ wo + bo

Sharding: data-parallel, 2 cores per batch image, each core owns 2048 of the
4096 queries but holds the full key/value set for its batch. No collectives.

Per-core pipeline (fp8 DoubleRow attention + dual-engine softmax exp):
  - xT [64, S] arrives chunk-major in bf16 and all small params packed in one
    [65, 258] f32 array, so every DMA line is a long contiguous HBM run and
    only a handful of DMA issues exist. x_q (residual) loads mid-loop so it
    never competes with xT for HBM bandwidth. There is no xT mirror at all:
    matmuls whose outputs belong on PSUM partitions 64:127 use
    tile_position=(0, 64) (column groups pick output partitions; inputs stay
    on rows 0:63).
  - GroupNorm stats via bn_stats/bn_aggr on DVE; tiny fp32 matmuls pair-
    combine channels to groups and expand back; quake-style rsqrt (one
    Newton step) on DVE keeps the scalar engine's activation tables pinned
    to exp. GN affine folds into the projection weights; k-bias is dropped
    (softmax-invariant), q-bias applied at the qT PSUM drain via the scalar
    engine's free per-partition bias. Throwaway matmuls gated on
    progressively later fold tiles keep the PE's HAM clock gate open through
    the stats/fold phase so stripe 0 runs at 2.4 GHz.
  - Key chunks are paired (2i, 2i+1): chunk 2i rides PE rows 0:63, 2i+1 rides
    rows 64:127, so the two K=64 score matmuls run concurrently. Scores land
    transposed ST[t, q] (keys on partitions) in one [128,1024] PSUM tile;
    THREE score buffers (6 of 8 PSUM banks) decouple the two exp engines.
    att@v runs ATTV_LAG pairs behind so its wait on exp never blocks later
    scores in the PE's strict-FIFO queue.
  - Softmax is max-free (|score/8| <= ~3, exp cannot overflow) and the exp of
    the scores is split across TWO engines working different key-pairs in
    parallel: ACT computes exp directly to fp8e4, while DVE computes a
    Schraudolph-style fast exp (bits = floor(log2e * s + 56.04) written as
    uint8 = the fp8e4 encoding of 2^(log2e*s/8)). Per-element error ~8% is
    random across the 4096 keys and averages out; attention weights are
    consistent numerator/denominator so softmax cancels any shared bias.
  - att@v runs in fp8 with perf_mode=DoubleRow: one matmul per key pair
    contracts all 256 keys (2 fp8 weights/cell), halving the PE streaming
    cost. v carries an appended exact-ones column so att@v also accumulates
    the softmax denominator l. A single [65,512] PSUM bank per query stripe
    accumulates att@v - no lo/hi halves, no merge pass.
  - The output projection runs on the unnormalized accumulator with an extra
    wo column passing l through; one reciprocal + fused multiply-add applies
    normalization, residual and bo. The v-bias enters via row 64 of wo_aug
    (bvo = (gnbias@Wv + bv)@wo), bounced through HBM to land on partition 64.
"""

import numpy as np
import ml_dtypes

import concourse.tile as tile
from concourse import bacc, mybir
from concourse.bass_utils import run_bass_kernel_spmd

F32 = mybir.dt.float32
BF16 = mybir.dt.bfloat16
FP8 = mybir.dt.float8e4
U8 = mybir.dt.uint8
U32 = mybir.dt.uint32
AF = mybir.ActivationFunctionType
ALU = mybir.AluOpType
PM = mybir.MatmulPerfMode

B, H, W, C = 4, 64, 64, 64
S = H * W             # 4096 spatial positions per image
SQ = S // 2           # 2048 queries per core
EPS = 1e-3
N_CHUNK = S // 128    # 32 key chunks
N_PAIR = N_CHUNK // 2  # 16 key pairs (2i, 2i+1)
N_STRIPE = SQ // 512  # 4 query stripes
SCALE = float(C) ** -0.5  # 0.125

# fast-exp constants: fp8e4(2^t) bits ~= 8*t + 56; t = SCALE*log2(e)*score.
# DVE computes bits = trunc(A8*score + B8C) in one tensor_scalar op; the
# uint8 result reinterpreted as fp8e4 is exp(score/8) to ~8% per element
# (bias-neutral calibration so ACT-pair and DVE-pair attention weights agree
# on average; errors are random across 4096 keys and average out).
A8 = float(np.log2(np.e))  # 8 * log2(e) * SCALE
B8C = 56.04                # 56 + truncation/staircase calibration

# packed param layout, f32 [65, 258]:
#   rows 0:64  cols 0:64 wq | 64:128 wk | 128:192 wv | 192 gamma |
#              193:257 wo | 257 beta
#   row  64    cols 0:64 bq | 128:192 bv | 193:257 bo
PK_COLS = 258

# exp engine assignment per (stripe, pair): True -> ACT, False -> DVE.
# Steady state is AADD: the score-PSUM pool has 2 buffers, so pair i+2 waits
# on exp(i) - AADD makes both the even and odd buffer chains alternate
# engines, keeping ACT and DVE exp running concurrently. Stripe 0 leans ACT
# (DVE is busy draining v tiles and the quad-1 projection).
_ACT_STRIPE0 = {0, 1, 2, 4, 6, 8, 10, 12, 14}


_ACT_STEADY = {1, 2, 4, 5, 7, 9, 11, 13, 15}


def _exp_on_act(j, i):
    if j == 0:
        return i in _ACT_STRIPE0
    return i in _ACT_STEADY


def build_kernel():
    nc = bacc.Bacc("TRN2", target_bir_lowering=False, debug=False)

    # xT chunk-major and x_q/out in on-chip layout so every DMA line is a
    # long contiguous HBM run (strided 2KB lines measured ~4x slower).
    xT_d = nc.dram_tensor("xT", [4, 64, 1024], BF16, kind="ExternalInput")
    xq_d = nc.dram_tensor("x_q", [128, SQ // 128, C], F32, kind="ExternalInput")
    pk_d = nc.dram_tensor("pk", [65, PK_COLS], F32, kind="ExternalInput")
    out_d = nc.dram_tensor("out", [128, SQ // 128, C], F32, kind="ExternalOutput")

    with tile.TileContext(nc) as tc:
        _emit(nc, tc, xT_d.ap(), xq_d.ap(), pk_d.ap(), out_d.ap())
    nc.compile()
    return nc


def _emit(nc, tc, xT_d, xq_d, pk_d, out_d):
    from contextlib import ExitStack

    ctx = ExitStack()
    with ctx:
        const = ctx.enter_context(tc.tile_pool(name="const", bufs=1))
        big = ctx.enter_context(tc.tile_pool(name="big", bufs=1))
        tiny = ctx.enter_context(tc.tile_pool(name="tiny", bufs=1))

        # ---- big input DMAs first; the scalar ring stays free for the
        # GroupNorm squares. No xT hi-half mirror exists at all: matmuls
        # that must write PSUM partitions 64:127 use tile_position=(0, 64)
        # (column groups select output partitions; inputs stay on rows
        # 0:63). x_q is NOT loaded here - it is only needed by the first
        # epilogue (~30us in) and would steal HBM bandwidth from xT. ----
        xT = big.tile([64, S], BF16)
        xq_sb = big.tile([128, SQ // 128, 64], F32)
        nc.sync.dma_start(out=xT[:, 0:1024], in_=xT_d[0])
        nc.scalar.dma_start(out=xT[:, 1024:2048], in_=xT_d[1])
        nc.gpsimd.dma_start(out=xT[:, 2048:3072], in_=xT_d[2])
        nc.sync.dma_start(out=xT[:, 3072:4096], in_=xT_d[3])

        zbias = const.tile([128, 1], F32)
        nc.gpsimd.memset(zbias, 0.0)
        # exp and square share one ACT table set; preload it while waiting
        # on input DMAs (rsqrt runs on DVE so no other set is ever needed).
        scratch1 = const.tile([1, 1], F32)
        nc.scalar.activation(scratch1, zbias[0:1, :], AF.Exp, bias=0.0, scale=1.0)

        # ---- packed params on the sync ring ----
        wqkv = const.tile([64, 193], F32)   # wq|wk|wv|gamma
        nc.sync.dma_start(out=wqkv, in_=pk_d[0:64, 0:193])
        wq_aug = const.tile([65, 64], F32)   # [Wq ; bq]
        nc.sync.dma_start(out=wq_aug, in_=pk_d[0:65, 0:64])
        wv_aug = const.tile([65, 65], F32)   # [Wv ; bv] plus e64 column
        nc.sync.dma_start(out=wv_aug[:, 0:64], in_=pk_d[0:65, 128:192])
        nc.gpsimd.memset(wv_aug[0:64, 64:65], 0.0)
        nc.gpsimd.memset(wv_aug[64:65, 64:65], 1.0)
        wo_sb = const.tile([64, 64], F32)
        nc.sync.dma_start(out=wo_sb, in_=pk_d[0:64, 193:257])
        # wo_aug = [wo ; bvo] plus e64 column that passes l through. Row 64
        # multiplies the l-row of the accumulator, so after the division by l
        # it contributes the constant row bvo = bv_total @ wo - this is how
        # the v-bias is applied without materializing it per-position.
        wo_aug = const.tile([65, 65], BF16)
        nc.gpsimd.dma_start(out=wo_aug[0:64, 0:64], in_=pk_d[0:64, 193:257])  # SWDGE casts
        nc.gpsimd.memset(wo_aug[0:64, 64:65], 0.0)
        nc.gpsimd.memset(wo_aug[64:65, 64:65], 1.0)
        beta_col = const.tile([64, 1], F32)
        nc.scalar.dma_start(out=beta_col, in_=pk_d[0:64, 257:258])
        bo_bcast = const.tile([128, 64], F32)
        nc.scalar.dma_start(out=bo_bcast, in_=pk_d[64:65, 193:257].to_broadcast([128, 64]))
        gamma_col = wqkv[:, 192:193]

        # pair matrices: p64h[c,g] = 0.5 iff c//2 == g ; p32x64[g,c] = 1 iff
        # c//2 == g. fp32 so the tiny stat matmuls take fp32 operands.
        p64h = const.tile([64, 32], F32)
        nc.gpsimd.memset(p64h, 0.5)
        nc.gpsimd.affine_select(out=p64h, in_=p64h, compare_op=ALU.is_ge,
                                fill=0.0, base=0, pattern=[[-2, 32]],
                                channel_multiplier=1)
        nc.gpsimd.affine_select(out=p64h, in_=p64h, compare_op=ALU.is_ge,
                                fill=0.0, base=1, pattern=[[2, 32]],
                                channel_multiplier=-1)
        p32x64 = const.tile([32, 64], F32)
        nc.gpsimd.memset(p32x64, 1.0)
        nc.gpsimd.affine_select(out=p32x64, in_=p32x64, compare_op=ALU.is_ge,
                                fill=0.0, base=0, pattern=[[1, 64]],
                                channel_multiplier=-2)
        nc.gpsimd.affine_select(out=p32x64, in_=p32x64, compare_op=ALU.is_ge,
                                fill=0.0, base=1, pattern=[[-1, 64]],
                                channel_multiplier=2)

        # ---- PSUM pools: st 3x[128,1024] = 6 banks, ot 1, aux 1 ----
        # 3 score buffers decouple the exp engines: pair i+3 waits exp(i),
        # so ACT and DVE exps of consecutive pairs run concurrently instead
        # of the 2-buffer ping-pong serializing score->exp->score.
        st_ps = ctx.enter_context(tc.tile_pool(name="st_ps", bufs=3, space="PSUM"))
        ot_ps = ctx.enter_context(tc.tile_pool(name="ot_ps", bufs=1, space="PSUM"))
        aux_ps = ctx.enter_context(tc.tile_pool(name="aux_ps", bufs=1, space="PSUM"))

        # ---- GroupNorm stats: bn_stats/bn_aggr on DVE, chunk-gated so each
        # op starts as its DMA lands ----
        bstats = tiny.tile([64, 8, 6], F32)
        for h in range(8):
            nc.vector.bn_stats(bstats[:, h, :], xT[:, 512 * h:512 * (h + 1)])
        # ---- PE warm-up: the HAM clock gate halves the PE clock after
        # ~3.4us idle, and the PE would sit idle through the whole stats/
        # fold phase. Keep it busy with throwaway matmuls gated on chunk 0
        # and then on progressively later fold tiles so the projection
        # quads and stripe 0 run at full clock. ----
        xTv = xT.rearrange("p (c k) -> p c k", k=128)  # [64, 32, 128]
        warm = st_ps.tile([128, 1024], F32, tag="st")
        for _ in range(10):
            nc.tensor.matmul(warm[:, 0:512], lhsT=xTv[:, 0, :],
                             rhs=xT[:, 0:512], tile_position=(0, 0))

        mv = tiny.tile([64, 2], F32)
        nc.vector.bn_aggr(mv, bstats)
        tot = tiny.tile([64, 2], F32)  # [mean_c, E[x^2]_c]
        nc.scalar.copy(out=tot[:, 0:1], in_=mv[:, 0:1])  # ACT, off DVE chain
        nc.vector.scalar_tensor_tensor(out=tot[:, 1:2], in0=mv[:, 0:1],
                                       scalar=mv[:, 0:1], in1=mv[:, 1:2],
                                       op0=ALU.mult, op1=ALU.add)
        for _ in range(2):
            nc.tensor.matmul(warm[0:32, 0:2], lhsT=p64h, rhs=tot,
                             tile_position=(0, 0))
        gpair = aux_ps.tile([32, 2], F32, tag="aux")  # group [mean, E[x^2]]
        nc.tensor.matmul(gpair, lhsT=p64h, rhs=tot)
        # rstd = rsqrt(var+eps) on DVE: quake bit-seed + one Newton step
        # (rel err ~2e-3; the GN scale tolerates it easily). packed32 col 1
        # is the group mean, copied from PSUM while the seed computes.
        packed32 = tiny.tile([32, 2], F32)        # [rstd_g | mean_g]
        gm = tiny.tile([32, 2], F32)
        nc.vector.tensor_copy(gm, gpair)
        nc.scalar.copy(out=packed32[:, 1:2], in_=gm[:, 0:1])  # mean, off DVE
        nv = tiny.tile([32, 1], F32)
        nc.vector.scalar_tensor_tensor(out=nv, in0=gm[:, 0:1],
                                       scalar=gm[:, 0:1], in1=gm[:, 1:2],
                                       op0=ALU.mult, op1=ALU.subtract)
        var = tiny.tile([32, 1], F32)
        nc.vector.tensor_scalar(out=var, in0=nv, scalar1=-1.0, scalar2=EPS,
                                op0=ALU.mult, op1=ALU.add)
        for _ in range(2):
            nc.tensor.matmul(warm[0:64, 0:1], lhsT=p32x64, rhs=var,
                             tile_position=(0, 0))
        magic = tiny.tile([32, 1], U32)
        nc.gpsimd.memset(magic, 0x5F3759DF)
        ybits = tiny.tile([32, 1], U32)
        nc.vector.tensor_scalar(out=ybits, in0=var.bitcast(U32), scalar1=1,
                                scalar2=None, op0=ALU.logical_shift_right)
        nc.vector.tensor_sub(ybits, magic, ybits)
        y = ybits.bitcast(F32)
        t2 = tiny.tile([32, 1], F32)
        nc.vector.scalar_tensor_tensor(out=t2, in0=y, scalar=var, in1=y,
                                       op0=ALU.mult, op1=ALU.mult)
        nc.vector.tensor_scalar(out=t2, in0=t2, scalar1=-0.5, scalar2=1.5,
                                op0=ALU.mult, op1=ALU.add)
        nc.vector.tensor_mul(packed32[:, 0:1], y, t2)
        for _ in range(2):
            nc.tensor.matmul(warm[0:64, 0:2], lhsT=p32x64, rhs=packed32,
                             tile_position=(0, 0))
        chan = aux_ps.tile([64, 2], F32, tag="aux")  # expand groups->channels
        nc.tensor.matmul(chan, lhsT=p32x64, rhs=packed32)
        scale_col = tiny.tile([64, 1], F32)       # rstd_g * gamma_c
        nc.vector.tensor_mul(scale_col, chan[:, 0:1], gamma_col)
        gnbias = tiny.tile([65, 1], F32)          # beta - mean*scale, aug 1
        nc.vector.tensor_mul(gnbias[0:64, :], chan[:, 1:2], scale_col)
        nc.vector.tensor_sub(gnbias[0:64, :], beta_col, gnbias[0:64, :])
        nc.gpsimd.memset(gnbias[64:65, :], 1.0)

        # ---- fold GN into the projection weights, one op for all three ----
        wsc = tiny.tile([64, 192], BF16)
        nc.vector.tensor_scalar_mul(wsc, wqkv[:, 0:192], scale_col)
        wq_sc = wsc[:, 0:64]
        wk_sc = wsc[:, 64:128]
        wv_sc = wsc[:, 128:192]

        bqp = aux_ps.tile([128, 1], F32, tag="aux")  # total q bias, both halves
        nc.tensor.matmul(bqp[0:64, :], lhsT=wq_aug, rhs=gnbias)
        nc.tensor.matmul(bqp[64:128, :], lhsT=wq_aug, rhs=gnbias,
                         tile_position=(0, 64))
        bq_col = tiny.tile([128, 1], F32)
        nc.scalar.copy(out=bq_col, in_=bqp)  # ACT: keeps the DVE queue clear
        bvo_stage = nc.dram_tensor("bvo_stage", [64], F32).ap()

        def make_bvo():
            # bvo row for wo_aug, bounced through HBM to land on partition
            # 64 (engines are lane-locked; DMA is not). Deferred into stripe
            # 0 - it only feeds the first epilogue, a stripe later - so its
            # DVE copies never sit ahead of the quad drains that gate the
            # first scores.
            bvcp = aux_ps.tile([65, 1], F32, tag="aux")
            nc.tensor.matmul(bvcp, lhsT=wv_aug, rhs=gnbias)
            bv_col = tiny.tile([64, 1], F32)
            nc.vector.tensor_copy(bv_col, bvcp[0:64, :])
            bvop = aux_ps.tile([1, 64], F32, tag="aux")
            nc.tensor.matmul(bvop, lhsT=bv_col, rhs=wo_sb)
            bvo_row = tiny.tile([1, 64], F32)
            nc.vector.tensor_copy(bvo_row, bvop)
            nc.sync.dma_start(out=bvo_stage.rearrange("(o c) -> o c", o=1), in_=bvo_row)
            nc.gpsimd.dma_start(out=wo_aug[64:65, 0:64],
                                in_=bvo_stage.rearrange("(o c) -> o c", o=1))

        # ---- residual base: x + bo (gpsimd; SBUF-only op); the x_q load
        # and this add are deferred into stripe 0 (see `deferred`) ----
        xb_sb = big.tile([128, SQ // 128, 64], F32)

        def load_xq():
            # scalar ring: its queue position (mid stripe 0) guarantees the
            # transfer cannot compete with the xT chunk loads
            nc.scalar.dma_start(out=xq_sb, in_=xq_d)

        def make_xb():
            nc.gpsimd.tensor_add(xb_sb, xq_sb,
                                 bo_bcast.rearrange("p (o c) -> p o c", o=1).broadcast_to([128, SQ // 128, 64]))

        # ---- k/q projections ----
        # kT layout: col block 128i holds chunk 2i on rows 0:63 and chunk
        # 2i+1 on rows 64:127 (pairs of adjacent chunks ride opposite PE
        # halves so score matmuls run concurrently and the fp8 att@v can
        # consume adjacent chunk pairs with DoubleRow). qT carries every
        # query on both halves. Quads are bank-staggered so concurrent
        # row-tiles never drain into the same bank. Quad-0 drains ride ACT
        # (idle before the first exp); quad-1 drains ride DVE.
        kT = big.tile([128, SQ], BF16)
        qT = big.tile([128, SQ], BF16)
        def k_quad(q, split):
            # all matmuls read rows 0:63; the odd-chunk ("hi") projections
            # land on PSUM partitions 64:127 via tile_position=(0, 64).
            # Col-half A is complete after 2 matmuls so its drain (and the
            # first scores) start early.
            g = st_ps.tile([128, 1024], F32, tag="st")
            c0 = 16 * q
            nc.tensor.matmul(g[0:64, 0:512], lhsT=wk_sc,
                             rhs=xTv[:, c0:c0 + 8:2, :], tile_position=(0, 0))
            nc.tensor.matmul(g[64:128, 0:512], lhsT=wk_sc,
                             rhs=xTv[:, c0 + 1:c0 + 8:2, :],
                             tile_position=(0, 64))
            nc.tensor.matmul(g[0:64, 512:1024], lhsT=wk_sc,
                             rhs=xTv[:, c0 + 8:c0 + 16:2, :], tile_position=(0, 0))
            nc.tensor.matmul(g[64:128, 512:1024], lhsT=wk_sc,
                             rhs=xTv[:, c0 + 9:c0 + 16:2, :],
                             tile_position=(0, 64))
            dst = kT[:, 1024 * q:1024 * (q + 1)]
            if split:
                nc.scalar.copy(out=dst[:, 0:512], in_=g[:, 0:512])
                nc.vector.tensor_copy(dst[:, 512:1024], g[:, 512:1024])
            else:
                nc.vector.tensor_copy(dst, g)

        def q_quad(q, split):
            g = st_ps.tile([128, 1024], F32, tag="st")
            lo = 1024 * q
            nc.tensor.matmul(g[0:64, 0:512], lhsT=wq_sc,
                             rhs=xT[:, lo:lo + 512], tile_position=(0, 0))
            nc.tensor.matmul(g[64:128, 0:512], lhsT=wq_sc,
                             rhs=xT[:, lo:lo + 512], tile_position=(0, 64))
            nc.tensor.matmul(g[0:64, 512:1024], lhsT=wq_sc,
                             rhs=xT[:, lo + 512:lo + 1024], tile_position=(0, 0))
            nc.tensor.matmul(g[64:128, 512:1024], lhsT=wq_sc,
                             rhs=xT[:, lo + 512:lo + 1024], tile_position=(0, 64))
            dst = qT[:, 1024 * q:1024 * (q + 1)]
            if split:
                nc.scalar.add(dst[:, 0:512], g[:, 0:512], bq_col)
                nc.vector.tensor_scalar_add(dst[:, 512:1024], g[:, 512:1024],
                                            bq_col)
            else:
                nc.scalar.add(dst, g, bq_col)   # Identity + per-partition bias

        k_quad(0, True)
        q_quad(0, True)

        # ---- v projection, natural [key, chunk, c] layout, fp8e4 ----
        # Column 64 = exact ones so att@v also accumulates the softmax
        # denominator l; chunk stride padded to 80B (DoubleRow weight AP
        # step must be 16B-aligned). Group g's two PSUM tiles ride the aux
        # bank and the (not-yet-allocated) ot bank; groups 1-3 and the
        # quad-1 projections are emitted a few pairs into stripe 0 so their
        # PE/DVE work interleaves with early scores instead of gating them.
        v_big = big.tile([128, N_CHUNK, 80], FP8)
        nc.gpsimd.memset(v_big[:, :, 64:65], 1.0)
        v8 = v_big.rearrange("p (g b) c -> p g b c", g=8)

        def v_group(g, act_cast=False):  # chunks 4g..4g+3
            vg = aux_ps.tile([128, 4, 64], F32, tag="aux")
            for b in range(4):
                nc.tensor.matmul(vg[:, b, :], lhsT=xTv[:, 4 * g + b, :],
                                 rhs=wv_sc, tile_position=(0, 0))
            if act_cast:
                nc.scalar.copy(out=v8[:, g, :, 0:64], in_=vg)
            else:
                nc.vector.tensor_copy(v8[:, g, :, 0:64], vg)

        v_group(0)

        # ---- main attention loop ----
        p_pool = ctx.enter_context(tc.tile_pool(name="p_pool", bufs=4))
        ep_pool = ctx.enter_context(tc.tile_pool(name="ep_pool", bufs=3))

        # Epilogue, split so the single ot PSUM read (one big copy) happens
        # right after the stripe's last att@v while the projection, the
        # softmax division (ALU divide on the otherwise-idle GPSIMD) and the
        # output DMA run later, off both exp engines.
        def make_epilogue(j, ot_sb):
            def epi():
                op = aux_ps.tile([128, 4, 65], F32, tag="aux")
                for m in range(4):
                    nc.tensor.matmul(op[:, m, :], lhsT=ot_sb[:, 128 * m:128 * (m + 1)],
                                     rhs=wo_aug)
                rl = ep_pool.tile([128, 4], F32, tag="rl", bufs=2)
                nc.vector.reciprocal(rl, op[:, :, 64:65].rearrange("p m o -> p (m o)"))
                res = ep_pool.tile([128, 4, 64], F32, tag="res", bufs=2)
                for m in range(4):
                    nc.vector.scalar_tensor_tensor(out=res[:, m, :],
                                                   in0=op[:, m, 0:64],
                                                   scalar=rl[:, m:m + 1],
                                                   in1=xb_sb[:, 4 * j + m, :],
                                                   op0=ALU.mult, op1=ALU.add)
                nc.sync.dma_start(out=out_d[:, 4 * j:4 * (j + 1), :], in_=res)
            return epi

        def last_epilogue(j, ot):
            # per-chunk pipeline to shorten the kernel tail: each 128-query
            # chunk drains, projects, normalizes and ships independently.
            op = aux_ps.tile([128, 4, 65], F32, tag="aux")
            for m in range(4):
                ots = ep_pool.tile([65, 128], BF16, tag="ots")
                src = ot[:, 128 * m:128 * (m + 1)]
                if m % 2 == 0:
                    nc.scalar.copy(out=ots, in_=src)
                else:
                    nc.vector.tensor_copy(ots, src)
                nc.tensor.matmul(op[:, m, :], lhsT=ots, rhs=wo_aug)
                rl = ep_pool.tile([128, 1], F32, tag="rl", bufs=2)
                nc.vector.reciprocal(rl, op[:, m, 64:65])
                res = ep_pool.tile([128, 64], F32, tag="lres", bufs=2)
                nc.vector.scalar_tensor_tensor(out=res, in0=op[:, m, 0:64],
                                               scalar=rl,
                                               in1=xb_sb[:, 4 * j + m, :],
                                               op0=ALU.mult, op1=ALU.add)
                ring = [nc.sync, nc.scalar, nc.sync, nc.gpsimd][m]
                ring.dma_start(out=out_d[:, 4 * j + m, :], in_=res)

        # att@v runs ATTV_LAG pairs behind the scores so its wait on exp
        # never blocks later score matmuls in the PE's strict-FIFO queue.
        # The per-stripe ot accumulator is allocated lazily at the first
        # att@v so deferred v-projection tiles can use the bank before it.
        ATTV_LAG = 3
        attv_q = []      # deque of (pt, i, j)
        stripe_ot = {}
        pending_epilogue = None
        v2 = v_big.rearrange("p (i two) c -> p i two c", two=2)  # [128,16,2,80]

        def emit_attv(pt, i, j):
            if j not in stripe_ot:
                stripe_ot[j] = ot_ps.tile([65, 512], F32, tag="ot",
                                          name=f"ot_{j}")
            nc.tensor.matmul(stripe_ot[j], lhsT=v2[:, i, :, 0:65], rhs=pt,
                             perf_mode=PM.DoubleRow, tile_position=(0, 0),
                             start=(i == 0), stop=(i == N_PAIR - 1))

        # work deferred into stripe 0, keyed by pair index: the remaining
        # v-projection groups (group g feeds att@v pair 2g+1), the quad-1
        # k/q projections and the residual base (x_q load + bo add)
        deferred = {0: lambda: v_group(1), 1: lambda: v_group(2, True),
                    2: lambda: v_group(3),
                    3: lambda: (v_group(4), k_quad(1, True)),
                    4: lambda: v_group(5, True),
                    5: lambda: (v_group(6), load_xq()),
                    6: lambda: v_group(7), 9: make_bvo, 12: make_xb}
        deferred1 = {2: lambda: q_quad(1, True)}  # q1 feeds stripes 2-3

        for j in range(N_STRIPE):
            for i in range(N_PAIR):
                st2 = st_ps.tile([128, 1024], F32, tag="st")
                nc.tensor.matmul(st2[:, 0:512],
                                 lhsT=kT[0:64, 128 * i:128 * (i + 1)],
                                 rhs=qT[0:64, 512 * j:512 * (j + 1)],
                                 tile_position=(0, 0))
                nc.tensor.matmul(st2[:, 512:1024],
                                 lhsT=kT[64:128, 128 * i:128 * (i + 1)],
                                 rhs=qT[64:128, 512 * j:512 * (j + 1)],
                                 tile_position=(64, 0))
                pt = p_pool.tile([128, 2, 512], FP8, tag="p", bufs=5)
                ptf = pt.rearrange("p a n -> p (a n)")
                if j == N_STRIPE - 1 and i == N_PAIR - 1:
                    # split the last exp across both engines: it gates the
                    # kernel tail
                    nc.scalar.activation(ptf[:, 0:512], st2[:, 0:512], AF.Exp,
                                         bias=zbias, scale=SCALE)
                    nc.vector.tensor_scalar(out=ptf[:, 512:1024].bitcast(U8),
                                            in0=st2[:, 512:1024], scalar1=A8,
                                            scalar2=B8C, op0=ALU.mult, op1=ALU.add)
                elif _exp_on_act(j, i):
                    nc.scalar.activation(ptf, st2, AF.Exp, bias=zbias, scale=SCALE)
                else:
                    nc.vector.tensor_scalar(out=ptf.bitcast(U8),
                                            in0=st2, scalar1=A8, scalar2=B8C,
                                            op0=ALU.mult, op1=ALU.add)
                if j == 0 and i in deferred:
                    deferred[i]()
                if j == 1 and i in deferred1:
                    deferred1[i]()
                if len(attv_q) >= ATTV_LAG:
                    args = attv_q.pop(0)
                    emit_attv(*args)
                    if args[1] == N_PAIR - 1:
                        # previous stripe complete: drain its accumulator on
                        # DVE (idle at stripe start - its first exp is pair 2)
                        ot_sb = ep_pool.tile([65, 512], BF16, tag="ot_sb", bufs=2)
                        nc.vector.tensor_copy(ot_sb, stripe_ot[args[2]])
                        pending_epilogue = make_epilogue(args[2], ot_sb)
                if i == 6 and pending_epilogue is not None:
                    pending_epilogue()
                    pending_epilogue = None
                attv_q.append((pt, i, j))
        for args in attv_q:
            emit_attv(*args)
        attv_q.clear()
        last_epilogue(N_STRIPE - 1, stripe_ot[N_STRIPE - 1])


_NC_CACHE = {}


def _get_nc():
    if "nc" not in _NC_CACHE:
        _NC_CACHE["nc"] = build_kernel()
    return _NC_CACHE["nc"]


def pack_params(gamma, beta, wq, bq, wk, wv, bv, wo, bo):
    pk = np.zeros((65, PK_COLS), np.float32)
    pk[0:64, 0:64] = np.asarray(wq, np.float32)
    pk[0:64, 64:128] = np.asarray(wk, np.float32)
    pk[0:64, 128:192] = np.asarray(wv, np.float32)
    pk[0:64, 192] = np.asarray(gamma, np.float32)
    pk[0:64, 193:257] = np.asarray(wo, np.float32)
    pk[0:64, 257] = np.asarray(beta, np.float32)
    pk[64, 0:64] = np.asarray(bq, np.float32)
    pk[64, 128:192] = np.asarray(bv, np.float32)
    pk[64, 193:257] = np.asarray(bo, np.float32)
    return pk


def build_in_maps(x, gamma, beta, wq, bq, wk, wv, bv, wo, bo):
    """Per-core NEFF input dicts plus (batch, rows) scatter info per core."""
    x = np.asarray(x, dtype=np.float32)
    pk = pack_params(gamma, beta, wq, bq, wk, wv, bv, wo, bo)
    xf = x.reshape(B, S, C)
    in_maps = []
    scatter = []
    for core in range(8):
        b, h = core // 2, core % 2
        own = slice(h * SQ, (h + 1) * SQ)
        other = slice((1 - h) * SQ, (2 - h) * SQ)
        x_local = np.concatenate([xf[b][own], xf[b][other]], axis=0)
        xT = x_local.T.astype(ml_dtypes.bfloat16)          # [64, 4096]
        xT = np.ascontiguousarray(xT.reshape(64, 4, 1024).transpose(1, 0, 2))
        xq = np.ascontiguousarray(
            x_local[:SQ].reshape(SQ // 128, 128, C).transpose(1, 0, 2))
        in_maps.append({"xT": xT, "x_q": xq, "pk": pk})
        scatter.append((b, np.arange(h * SQ, (h + 1) * SQ)))
    return in_maps, scatter


def _run(in_maps, scatter, **spmd_kwargs):
    nc = _get_nc()
    res = run_bass_kernel_spmd(nc, in_maps, core_ids=list(range(8)),
                               **spmd_kwargs)
    out = np.empty((B, S, C), np.float32)
    for core in range(8):
        b, rows = scatter[core]
        r = res.results[core]["out"]  # [128, SQ//128, C] device layout
        out[b][rows] = r.transpose(1, 0, 2).reshape(SQ, C)
    return out.reshape(B, H, W, C), res


def kernel(x, gamma, beta, wq, bq, wk, bk, wv, bv, wo, bo):
    # bk is provably a no-op: it shifts each query's scores by a constant
    # which softmax cancels, so it is not shipped to the device.
    in_maps, scatter = build_in_maps(x, gamma, beta, wq, bq, wk, wv, bv, wo, bo)
    try:
        out, _ = _run(in_maps, scatter)
    except Exception:
        # transient NRT device hiccups have been observed on a first run;
        # one retry has always succeeded
        out, _ = _run(in_maps, scatter)
    return out


# revision 91
# speedup vs baseline: 1.0375x; 1.0375x over previous
"""Trainium2 Bass kernel for nn_AttentionBlock (B=4, H=W=64, C=64, GROUPS=32).

Math (reference):
    hn = GroupNorm(x; gamma, beta, 32 groups, eps=1e-3)
    q = hn@wq+bq ; k = hn@wk+bk ; v = hn@wv+bv
    att = softmax(q k^T / 8) over the 4096 spatial positions
    out = x + (att @ v) @ wo + bo

Sharding: data-parallel, 2 cores per batch image, each core owns 2048 of the
4096 queries but holds the full key/value set for its batch. No collectives.

Per-core pipeline (fp8 DoubleRow attention + dual-engine softmax exp):
  - xT [64, S] arrives chunk-major in bf16 and all small params packed in one
    [65, 258] f32 array, so every DMA line is a long contiguous HBM run and
    only a handful of DMA issues exist. x_q (residual) loads mid-loop so it
    never competes with xT for HBM bandwidth. There is no xT mirror at all:
    matmuls whose outputs belong on PSUM partitions 64:127 use
    tile_position=(0, 64) (column groups pick output partitions; inputs stay
    on rows 0:63).
  - GroupNorm stats via bn_stats/bn_aggr on DVE; tiny fp32 matmuls pair-
    combine channels to groups and expand back; quake-style rsqrt (one
    Newton step) on DVE keeps the scalar engine's activation tables pinned
    to exp. GN affine folds into the projection weights; k-bias is dropped
    (softmax-invariant), q-bias applied at the qT PSUM drain via the scalar
    engine's free per-partition bias. Throwaway matmuls gated on
    progressively later fold tiles keep the PE's HAM clock gate open through
    the stats/fold phase so stripe 0 runs at 2.4 GHz.
  - Key chunks are paired (2i, 2i+1): chunk 2i rides PE rows 0:63, 2i+1 rides
    rows 64:127, so the two K=64 score matmuls run concurrently. Scores land
    transposed ST[t, q] (keys on partitions) in one [128,1024] PSUM tile;
    THREE score buffers (6 of 8 PSUM banks) decouple the two exp engines.
    att@v runs ATTV_LAG pairs behind so its wait on exp never blocks later
    scores in the PE's strict-FIFO queue.
  - Softmax is max-free (|score/8| <= ~3, exp cannot overflow) and the exp of
    the scores is split across TWO engines working different key-pairs in
    parallel: ACT computes exp directly to fp8e4, while DVE computes a
    Schraudolph-style fast exp (bits = floor(log2e * s + 56.04) written as
    uint8 = the fp8e4 encoding of 2^(log2e*s/8)). Per-element error ~8% is
    random across the 4096 keys and averages out; attention weights are
    consistent numerator/denominator so softmax cancels any shared bias.
  - att@v runs in fp8 with perf_mode=DoubleRow: one matmul per key pair
    contracts all 256 keys (2 fp8 weights/cell), halving the PE streaming
    cost. v carries an appended exact-ones column so att@v also accumulates
    the softmax denominator l. A single [65,512] PSUM bank per query stripe
    accumulates att@v - no lo/hi halves, no merge pass.
  - The output projection runs on the unnormalized accumulator with an extra
    wo column passing l through; one reciprocal + fused multiply-add applies
    normalization, residual and bo. The v-bias enters via row 64 of wo_aug
    (bvo = (gnbias@Wv + bv)@wo), bounced through HBM to land on partition 64.
"""

import numpy as np
import ml_dtypes

import concourse.tile as tile
from concourse import bacc, mybir
from concourse.bass_utils import run_bass_kernel_spmd

F32 = mybir.dt.float32
BF16 = mybir.dt.bfloat16
FP8 = mybir.dt.float8e4
U8 = mybir.dt.uint8
U32 = mybir.dt.uint32
AF = mybir.ActivationFunctionType
ALU = mybir.AluOpType
PM = mybir.MatmulPerfMode

B, H, W, C = 4, 64, 64, 64
S = H * W             # 4096 spatial positions per image
SQ = S // 2           # 2048 queries per core
EPS = 1e-3
N_CHUNK = S // 128    # 32 key chunks
N_PAIR = N_CHUNK // 2  # 16 key pairs (2i, 2i+1)
N_STRIPE = SQ // 512  # 4 query stripes
SCALE = float(C) ** -0.5  # 0.125

# fast-exp constants: fp8e4(2^t) bits ~= 8*t + 56; t = SCALE*log2(e)*score.
# DVE computes bits = trunc(A8*score + B8C) in one tensor_scalar op; the
# uint8 result reinterpreted as fp8e4 is exp(score/8) to ~8% per element
# (bias-neutral calibration so ACT-pair and DVE-pair attention weights agree
# on average; errors are random across 4096 keys and average out).
A8 = float(np.log2(np.e))  # 8 * log2(e) * SCALE
B8C = 56.04                # 56 + truncation/staircase calibration

# packed param layout, f32 [65, 258]:
#   rows 0:64  cols 0:64 wq | 64:128 wk | 128:192 wv | 192 gamma |
#              193:257 wo | 257 beta
#   row  64    cols 0:64 bq | 128:192 bv | 193:257 bo
PK_COLS = 258

# exp engine assignment per (stripe, pair): True -> ACT, False -> DVE.
# Steady state is AADD: the score-PSUM pool has 2 buffers, so pair i+2 waits
# on exp(i) - AADD makes both the even and odd buffer chains alternate
# engines, keeping ACT and DVE exp running concurrently. Stripe 0 leans ACT
# (DVE is busy draining v tiles and the quad-1 projection).
_ACT_STRIPE0 = {0, 1, 2, 4, 6, 8, 10, 12, 14}


_ACT_STEADY = {1, 2, 4, 5, 7, 9, 11, 13, 15}


def _exp_on_act(j, i):
    if j == 0:
        return i in _ACT_STRIPE0
    return i in _ACT_STEADY


def build_kernel():
    nc = bacc.Bacc("TRN2", target_bir_lowering=False, debug=False)

    # xT chunk-major and x_q/out in on-chip layout so every DMA line is a
    # long contiguous HBM run (strided 2KB lines measured ~4x slower).
    xT_d = nc.dram_tensor("xT", [4, 64, 1024], BF16, kind="ExternalInput")
    xq_d = nc.dram_tensor("x_q", [128, SQ // 128, C], F32, kind="ExternalInput")
    pk_d = nc.dram_tensor("pk", [65, PK_COLS], F32, kind="ExternalInput")
    out_d = nc.dram_tensor("out", [128, SQ // 128, C], F32, kind="ExternalOutput")

    with tile.TileContext(nc) as tc:
        _emit(nc, tc, xT_d.ap(), xq_d.ap(), pk_d.ap(), out_d.ap())
    nc.compile()
    return nc


def _emit(nc, tc, xT_d, xq_d, pk_d, out_d):
    from contextlib import ExitStack

    ctx = ExitStack()
    with ctx:
        const = ctx.enter_context(tc.tile_pool(name="const", bufs=1))
        big = ctx.enter_context(tc.tile_pool(name="big", bufs=1))
        tiny = ctx.enter_context(tc.tile_pool(name="tiny", bufs=1))

        # ---- big input DMAs first; the scalar ring stays free for the
        # GroupNorm squares. No xT hi-half mirror exists at all: matmuls
        # that must write PSUM partitions 64:127 use tile_position=(0, 64)
        # (column groups select output partitions; inputs stay on rows
        # 0:63). x_q is NOT loaded here - it is only needed by the first
        # epilogue (~30us in) and would steal HBM bandwidth from xT. ----
        xT = big.tile([64, S], BF16)
        xq_sb = big.tile([128, SQ // 128, 64], F32)
        # chunk 1 rides the gpsimd ring: the scalar ring's hoisted
        # ACT_TABLE_LOAD delays its first DMA, and the serial bn_stats chain
        # consumes chunks in order, so the later-needed chunk 2 absorbs it
        nc.sync.dma_start(out=xT[:, 0:1024], in_=xT_d[0])
        nc.gpsimd.dma_start(out=xT[:, 1024:2048], in_=xT_d[1])
        nc.scalar.dma_start(out=xT[:, 2048:3072], in_=xT_d[2])
        nc.sync.dma_start(out=xT[:, 3072:4096], in_=xT_d[3])

        zbias = const.tile([128, 1], F32)
        nc.gpsimd.memset(zbias, 0.0)
        # exp and square share one ACT table set; preload it while waiting
        # on input DMAs (rsqrt runs on DVE so no other set is ever needed).
        scratch1 = const.tile([1, 1], F32)
        nc.scalar.activation(scratch1, zbias[0:1, :], AF.Exp, bias=0.0, scale=1.0)

        # ---- packed params on the sync ring ----
        wqkv = const.tile([64, 193], F32)   # wq|wk|wv|gamma
        nc.sync.dma_start(out=wqkv, in_=pk_d[0:64, 0:193])
        wq_aug = const.tile([65, 64], F32)   # [Wq ; bq]
        nc.sync.dma_start(out=wq_aug, in_=pk_d[0:65, 0:64])
        wv_aug = const.tile([65, 65], F32)   # [Wv ; bv] plus e64 column
        nc.sync.dma_start(out=wv_aug[:, 0:64], in_=pk_d[0:65, 128:192])
        nc.gpsimd.memset(wv_aug[0:64, 64:65], 0.0)
        nc.gpsimd.memset(wv_aug[64:65, 64:65], 1.0)
        wo_sb = const.tile([64, 64], F32)
        nc.sync.dma_start(out=wo_sb, in_=pk_d[0:64, 193:257])
        # wo_aug = [wo ; bvo] plus e64 column that passes l through. Row 64
        # multiplies the l-row of the accumulator, so after the division by l
        # it contributes the constant row bvo = bv_total @ wo - this is how
        # the v-bias is applied without materializing it per-position.
        wo_aug = const.tile([65, 65], BF16)
        nc.gpsimd.dma_start(out=wo_aug[0:64, 0:64], in_=pk_d[0:64, 193:257])  # SWDGE casts
        nc.gpsimd.memset(wo_aug[0:64, 64:65], 0.0)
        nc.gpsimd.memset(wo_aug[64:65, 64:65], 1.0)
        beta_col = const.tile([64, 1], F32)
        nc.scalar.dma_start(out=beta_col, in_=pk_d[0:64, 257:258])
        bo_bcast = const.tile([128, 64], F32)
        nc.scalar.dma_start(out=bo_bcast, in_=pk_d[64:65, 193:257].to_broadcast([128, 64]))
        gamma_col = wqkv[:, 192:193]

        # pair matrices: p64h[c,g] = 0.5 iff c//2 == g ; p32x64[g,c] = 1 iff
        # c//2 == g. fp32 so the tiny stat matmuls take fp32 operands.
        p64h = const.tile([64, 32], F32)
        nc.gpsimd.memset(p64h, 0.5)
        nc.gpsimd.affine_select(out=p64h, in_=p64h, compare_op=ALU.is_ge,
                                fill=0.0, base=0, pattern=[[-2, 32]],
                                channel_multiplier=1)
        nc.gpsimd.affine_select(out=p64h, in_=p64h, compare_op=ALU.is_ge,
                                fill=0.0, base=1, pattern=[[2, 32]],
                                channel_multiplier=-1)
        p32x64 = const.tile([32, 64], F32)
        nc.gpsimd.memset(p32x64, 1.0)
        nc.gpsimd.affine_select(out=p32x64, in_=p32x64, compare_op=ALU.is_ge,
                                fill=0.0, base=0, pattern=[[1, 64]],
                                channel_multiplier=-2)
        nc.gpsimd.affine_select(out=p32x64, in_=p32x64, compare_op=ALU.is_ge,
                                fill=0.0, base=1, pattern=[[-1, 64]],
                                channel_multiplier=2)

        # ---- PSUM pools: st 3x[128,1024] = 6 banks, ot 1, aux 1 ----
        # 3 score buffers decouple the exp engines: pair i+3 waits exp(i),
        # so ACT and DVE exps of consecutive pairs run concurrently instead
        # of the 2-buffer ping-pong serializing score->exp->score.
        st_ps = ctx.enter_context(tc.tile_pool(name="st_ps", bufs=3, space="PSUM"))
        ot_ps = ctx.enter_context(tc.tile_pool(name="ot_ps", bufs=1, space="PSUM"))
        aux_ps = ctx.enter_context(tc.tile_pool(name="aux_ps", bufs=1, space="PSUM"))

        # ---- GroupNorm stats: bn_stats/bn_aggr on DVE, chunk-gated so each
        # op starts as its DMA lands ----
        bstats = tiny.tile([64, 8, 6], F32)
        for h in range(8):
            nc.vector.bn_stats(bstats[:, h, :], xT[:, 512 * h:512 * (h + 1)])
        # ---- PE warm-up: the HAM clock gate halves the PE clock after
        # ~3.4us idle, and the PE would sit idle through the whole stats/
        # fold phase. Keep it busy with throwaway matmuls gated on chunk 0
        # and then on progressively later fold tiles so the projection
        # quads and stripe 0 run at full clock. ----
        xTv = xT.rearrange("p (c k) -> p c k", k=128)  # [64, 32, 128]
        warm = st_ps.tile([128, 1024], F32, tag="st")
        for _ in range(10):
            nc.tensor.matmul(warm[:, 0:512], lhsT=xTv[:, 0, :],
                             rhs=xT[:, 0:512], tile_position=(0, 0))

        mv = tiny.tile([64, 2], F32)
        nc.vector.bn_aggr(mv, bstats)
        tot = tiny.tile([64, 2], F32)  # [mean_c, E[x^2]_c]
        nc.scalar.copy(out=tot[:, 0:1], in_=mv[:, 0:1])  # ACT, off DVE chain
        nc.vector.scalar_tensor_tensor(out=tot[:, 1:2], in0=mv[:, 0:1],
                                       scalar=mv[:, 0:1], in1=mv[:, 1:2],
                                       op0=ALU.mult, op1=ALU.add)
        for _ in range(2):
            nc.tensor.matmul(warm[0:32, 0:2], lhsT=p64h, rhs=tot,
                             tile_position=(0, 0))
        gpair = aux_ps.tile([32, 2], F32, tag="aux")  # group [mean, E[x^2]]
        nc.tensor.matmul(gpair, lhsT=p64h, rhs=tot)
        # rstd = rsqrt(var+eps) on DVE: quake bit-seed + one Newton step
        # (rel err ~2e-3; the GN scale tolerates it easily). packed32 col 1
        # is the group mean, copied from PSUM while the seed computes.
        packed32 = tiny.tile([32, 2], F32)        # [rstd_g | mean_g]
        gm = tiny.tile([32, 2], F32)
        nc.vector.tensor_copy(gm, gpair)
        nc.scalar.copy(out=packed32[:, 1:2], in_=gm[:, 0:1])  # mean, off DVE
        nv = tiny.tile([32, 1], F32)
        nc.vector.scalar_tensor_tensor(out=nv, in0=gm[:, 0:1],
                                       scalar=gm[:, 0:1], in1=gm[:, 1:2],
                                       op0=ALU.mult, op1=ALU.subtract)
        var = tiny.tile([32, 1], F32)
        nc.vector.tensor_scalar(out=var, in0=nv, scalar1=-1.0, scalar2=EPS,
                                op0=ALU.mult, op1=ALU.add)
        for _ in range(2):
            nc.tensor.matmul(warm[0:64, 0:1], lhsT=p32x64, rhs=var,
                             tile_position=(0, 0))
        magic = tiny.tile([32, 1], U32)
        nc.gpsimd.memset(magic, 0x5F3759DF)
        ybits = tiny.tile([32, 1], U32)
        nc.vector.tensor_scalar(out=ybits, in0=var.bitcast(U32), scalar1=1,
                                scalar2=None, op0=ALU.logical_shift_right)
        nc.vector.tensor_sub(ybits, magic, ybits)
        y = ybits.bitcast(F32)
        t2 = tiny.tile([32, 1], F32)
        nc.vector.scalar_tensor_tensor(out=t2, in0=y, scalar=var, in1=y,
                                       op0=ALU.mult, op1=ALU.mult)
        nc.vector.tensor_scalar(out=t2, in0=t2, scalar1=-0.5, scalar2=1.5,
                                op0=ALU.mult, op1=ALU.add)
        nc.vector.tensor_mul(packed32[:, 0:1], y, t2)
        for _ in range(2):
            nc.tensor.matmul(warm[0:64, 0:2], lhsT=p32x64, rhs=packed32,
                             tile_position=(0, 0))
        chan = aux_ps.tile([64, 2], F32, tag="aux")  # expand groups->channels
        nc.tensor.matmul(chan, lhsT=p32x64, rhs=packed32)
        scale_col = tiny.tile([64, 1], F32)       # rstd_g * gamma_c
        nc.vector.tensor_mul(scale_col, chan[:, 0:1], gamma_col)
        gnbias = tiny.tile([65, 1], F32)          # beta - mean*scale, aug 1
        nc.vector.tensor_mul(gnbias[0:64, :], chan[:, 1:2], scale_col)
        nc.vector.tensor_sub(gnbias[0:64, :], beta_col, gnbias[0:64, :])
        nc.gpsimd.memset(gnbias[64:65, :], 1.0)

        # ---- fold GN into the projection weights, one op for all three ----
        wsc = tiny.tile([64, 192], BF16)
        nc.vector.tensor_scalar_mul(wsc, wqkv[:, 0:192], scale_col)
        wq_sc = wsc[:, 0:64]
        wk_sc = wsc[:, 64:128]
        wv_sc = wsc[:, 128:192]

        bqp = aux_ps.tile([128, 1], F32, tag="aux")  # total q bias, both halves
        nc.tensor.matmul(bqp[0:64, :], lhsT=wq_aug, rhs=gnbias)
        nc.tensor.matmul(bqp[64:128, :], lhsT=wq_aug, rhs=gnbias,
                         tile_position=(0, 64))
        bq_col = tiny.tile([128, 1], F32)
        nc.scalar.copy(out=bq_col, in_=bqp)  # ACT: keeps the DVE queue clear
        bvo_stage = nc.dram_tensor("bvo_stage", [64], F32).ap()

        def make_bvo():
            # bvo row for wo_aug, bounced through HBM to land on partition
            # 64 (engines are lane-locked; DMA is not). Deferred into stripe
            # 0 - it only feeds the first epilogue, a stripe later - so its
            # DVE copies never sit ahead of the quad drains that gate the
            # first scores.
            bvcp = aux_ps.tile([65, 1], F32, tag="aux")
            nc.tensor.matmul(bvcp, lhsT=wv_aug, rhs=gnbias)
            bv_col = tiny.tile([64, 1], F32)
            nc.vector.tensor_copy(bv_col, bvcp[0:64, :])
            bvop = aux_ps.tile([1, 64], F32, tag="aux")
            nc.tensor.matmul(bvop, lhsT=bv_col, rhs=wo_sb)
            bvo_row = tiny.tile([1, 64], F32)
            nc.vector.tensor_copy(bvo_row, bvop)
            nc.sync.dma_start(out=bvo_stage.rearrange("(o c) -> o c", o=1), in_=bvo_row)
            nc.gpsimd.dma_start(out=wo_aug[64:65, 0:64],
                                in_=bvo_stage.rearrange("(o c) -> o c", o=1))

        # ---- residual base: x + bo (gpsimd; SBUF-only op); the x_q load
        # and this add are deferred into stripe 0 (see `deferred`) ----
        xb_sb = big.tile([128, SQ // 128, 64], F32)

        def load_xq():
            # scalar ring: its queue position (mid stripe 0) guarantees the
            # transfer cannot compete with the xT chunk loads
            nc.scalar.dma_start(out=xq_sb, in_=xq_d)

        def make_xb():
            nc.gpsimd.tensor_add(xb_sb, xq_sb,
                                 bo_bcast.rearrange("p (o c) -> p o c", o=1).broadcast_to([128, SQ // 128, 64]))

        # ---- k/q projections ----
        # kT layout: col block 128i holds chunk 2i on rows 0:63 and chunk
        # 2i+1 on rows 64:127 (pairs of adjacent chunks ride opposite PE
        # halves so score matmuls run concurrently and the fp8 att@v can
        # consume adjacent chunk pairs with DoubleRow). qT carries every
        # query on both halves. Quads are bank-staggered so concurrent
        # row-tiles never drain into the same bank. Quad-0 drains ride ACT
        # (idle before the first exp); quad-1 drains ride DVE.
        kT = big.tile([128, SQ], BF16)
        qT = big.tile([128, SQ], BF16)
        def k_quad(q, split):
            # all matmuls read rows 0:63; the odd-chunk ("hi") projections
            # land on PSUM partitions 64:127 via tile_position=(0, 64).
            # Col-half A is complete after 2 matmuls so its drain (and the
            # first scores) start early.
            g = st_ps.tile([128, 1024], F32, tag="st")
            c0 = 16 * q
            nc.tensor.matmul(g[0:64, 0:512], lhsT=wk_sc,
                             rhs=xTv[:, c0:c0 + 8:2, :], tile_position=(0, 0))
            nc.tensor.matmul(g[64:128, 0:512], lhsT=wk_sc,
                             rhs=xTv[:, c0 + 1:c0 + 8:2, :],
                             tile_position=(0, 64))
            nc.tensor.matmul(g[0:64, 512:1024], lhsT=wk_sc,
                             rhs=xTv[:, c0 + 8:c0 + 16:2, :], tile_position=(0, 0))
            nc.tensor.matmul(g[64:128, 512:1024], lhsT=wk_sc,
                             rhs=xTv[:, c0 + 9:c0 + 16:2, :],
                             tile_position=(0, 64))
            dst = kT[:, 1024 * q:1024 * (q + 1)]
            if split:
                nc.scalar.copy(out=dst[:, 0:512], in_=g[:, 0:512])
                nc.vector.tensor_copy(dst[:, 512:1024], g[:, 512:1024])
            else:
                nc.vector.tensor_copy(dst, g)

        def q_quad(q, split):
            g = st_ps.tile([128, 1024], F32, tag="st")
            lo = 1024 * q
            nc.tensor.matmul(g[0:64, 0:512], lhsT=wq_sc,
                             rhs=xT[:, lo:lo + 512], tile_position=(0, 0))
            nc.tensor.matmul(g[64:128, 0:512], lhsT=wq_sc,
                             rhs=xT[:, lo:lo + 512], tile_position=(0, 64))
            nc.tensor.matmul(g[0:64, 512:1024], lhsT=wq_sc,
                             rhs=xT[:, lo + 512:lo + 1024], tile_position=(0, 0))
            nc.tensor.matmul(g[64:128, 512:1024], lhsT=wq_sc,
                             rhs=xT[:, lo + 512:lo + 1024], tile_position=(0, 64))
            dst = qT[:, 1024 * q:1024 * (q + 1)]
            if split:
                nc.scalar.add(dst[:, 0:512], g[:, 0:512], bq_col)
                nc.vector.tensor_scalar_add(dst[:, 512:1024], g[:, 512:1024],
                                            bq_col)
            else:
                nc.scalar.add(dst, g, bq_col)   # Identity + per-partition bias

        k_quad(0, True)
        q_quad(0, True)

        # ---- v projection, natural [key, chunk, c] layout, fp8e4 ----
        # Column 64 = exact ones so att@v also accumulates the softmax
        # denominator l; chunk stride padded to 80B (DoubleRow weight AP
        # step must be 16B-aligned). Group g's two PSUM tiles ride the aux
        # bank and the (not-yet-allocated) ot bank; groups 1-3 and the
        # quad-1 projections are emitted a few pairs into stripe 0 so their
        # PE/DVE work interleaves with early scores instead of gating them.
        v_big = big.tile([128, N_CHUNK, 80], FP8)
        nc.gpsimd.memset(v_big[:, :, 64:65], 1.0)
        v8 = v_big.rearrange("p (g b) c -> p g b c", g=8)

        def v_group(g, act_cast=False):  # chunks 4g..4g+3
            vg = aux_ps.tile([128, 4, 64], F32, tag="aux")
            for b in range(4):
                nc.tensor.matmul(vg[:, b, :], lhsT=xTv[:, 4 * g + b, :],
                                 rhs=wv_sc, tile_position=(0, 0))
            if act_cast:
                nc.scalar.copy(out=v8[:, g, :, 0:64], in_=vg)
            else:
                nc.vector.tensor_copy(v8[:, g, :, 0:64], vg)

        v_group(0)

        # ---- main attention loop ----
        p_pool = ctx.enter_context(tc.tile_pool(name="p_pool", bufs=4))
        ep_pool = ctx.enter_context(tc.tile_pool(name="ep_pool", bufs=3))

        # Epilogue, split so the single ot PSUM read (one big copy) happens
        # right after the stripe's last att@v while the projection, the
        # softmax division (ALU divide on the otherwise-idle GPSIMD) and the
        # output DMA run later, off both exp engines.
        def make_epilogue(j, ot_sb):
            def epi():
                op = aux_ps.tile([128, 4, 65], F32, tag="aux")
                for m in range(4):
                    nc.tensor.matmul(op[:, m, :], lhsT=ot_sb[:, 128 * m:128 * (m + 1)],
                                     rhs=wo_aug)
                rl = ep_pool.tile([128, 4], F32, tag="rl", bufs=2)
                nc.vector.reciprocal(rl, op[:, :, 64:65].rearrange("p m o -> p (m o)"))
                res = ep_pool.tile([128, 4, 64], F32, tag="res", bufs=2)
                for m in range(4):
                    nc.vector.scalar_tensor_tensor(out=res[:, m, :],
                                                   in0=op[:, m, 0:64],
                                                   scalar=rl[:, m:m + 1],
                                                   in1=xb_sb[:, 4 * j + m, :],
                                                   op0=ALU.mult, op1=ALU.add)
                nc.sync.dma_start(out=out_d[:, 4 * j:4 * (j + 1), :], in_=res)
            return epi

        def last_epilogue(j, ot):
            # per-chunk pipeline to shorten the kernel tail: each 128-query
            # chunk drains, projects, normalizes and ships independently.
            op = aux_ps.tile([128, 4, 65], F32, tag="aux")
            for m in range(4):
                ots = ep_pool.tile([65, 128], BF16, tag="ots")
                src = ot[:, 128 * m:128 * (m + 1)]
                if m % 2 == 0:
                    nc.scalar.copy(out=ots, in_=src)
                else:
                    nc.vector.tensor_copy(ots, src)
                nc.tensor.matmul(op[:, m, :], lhsT=ots, rhs=wo_aug)
                rl = ep_pool.tile([128, 1], F32, tag="rl", bufs=2)
                nc.vector.reciprocal(rl, op[:, m, 64:65])
                res = ep_pool.tile([128, 64], F32, tag="lres", bufs=2)
                nc.vector.scalar_tensor_tensor(out=res, in0=op[:, m, 0:64],
                                               scalar=rl,
                                               in1=xb_sb[:, 4 * j + m, :],
                                               op0=ALU.mult, op1=ALU.add)
                ring = [nc.sync, nc.scalar, nc.sync, nc.gpsimd][m]
                ring.dma_start(out=out_d[:, 4 * j + m, :], in_=res)

        # att@v runs ATTV_LAG pairs behind the scores so its wait on exp
        # never blocks later score matmuls in the PE's strict-FIFO queue.
        # The per-stripe ot accumulator is allocated lazily at the first
        # att@v so deferred v-projection tiles can use the bank before it.
        ATTV_LAG = 3
        attv_q = []      # deque of (pt, i, j)
        stripe_ot = {}
        pending_epilogue = None
        v2 = v_big.rearrange("p (i two) c -> p i two c", two=2)  # [128,16,2,80]

        def emit_attv(pt, i, j):
            if j not in stripe_ot:
                stripe_ot[j] = ot_ps.tile([65, 512], F32, tag="ot",
                                          name=f"ot_{j}")
            nc.tensor.matmul(stripe_ot[j], lhsT=v2[:, i, :, 0:65], rhs=pt,
                             perf_mode=PM.DoubleRow, tile_position=(0, 0),
                             start=(i == 0), stop=(i == N_PAIR - 1))

        # work deferred into stripe 0, keyed by pair index: the remaining
        # v-projection groups (group g feeds att@v pair 2g+1), the quad-1
        # k/q projections and the residual base (x_q load + bo add)
        deferred = {0: lambda: v_group(1), 1: lambda: v_group(2, True),
                    2: lambda: v_group(3),
                    3: lambda: (v_group(4), k_quad(1, True)),
                    4: lambda: v_group(5, True),
                    5: lambda: (v_group(6), load_xq()),
                    6: lambda: v_group(7), 9: make_bvo, 12: make_xb}
        deferred1 = {5: lambda: q_quad(1, True)}  # q1 feeds stripes 2-3

        for j in range(N_STRIPE):
            for i in range(N_PAIR):
                st2 = st_ps.tile([128, 1024], F32, tag="st")
                nc.tensor.matmul(st2[:, 0:512],
                                 lhsT=kT[0:64, 128 * i:128 * (i + 1)],
                                 rhs=qT[0:64, 512 * j:512 * (j + 1)],
                                 tile_position=(0, 0))
                nc.tensor.matmul(st2[:, 512:1024],
                                 lhsT=kT[64:128, 128 * i:128 * (i + 1)],
                                 rhs=qT[64:128, 512 * j:512 * (j + 1)],
                                 tile_position=(64, 0))
                pt = p_pool.tile([128, 2, 512], FP8, tag="p", bufs=5)
                ptf = pt.rearrange("p a n -> p (a n)")
                if j == N_STRIPE - 1 and i == N_PAIR - 1:
                    # split the last exp across both engines: it gates the
                    # kernel tail
                    nc.scalar.activation(ptf[:, 0:512], st2[:, 0:512], AF.Exp,
                                         bias=zbias, scale=SCALE)
                    nc.vector.tensor_scalar(out=ptf[:, 512:1024].bitcast(U8),
                                            in0=st2[:, 512:1024], scalar1=A8,
                                            scalar2=B8C, op0=ALU.mult, op1=ALU.add)
                elif _exp_on_act(j, i):
                    nc.scalar.activation(ptf, st2, AF.Exp, bias=zbias, scale=SCALE)
                else:
                    nc.vector.tensor_scalar(out=ptf.bitcast(U8),
                                            in0=st2, scalar1=A8, scalar2=B8C,
                                            op0=ALU.mult, op1=ALU.add)
                if j == 0 and i in deferred:
                    deferred[i]()
                if j == 1 and i in deferred1:
                    deferred1[i]()
                if len(attv_q) >= ATTV_LAG:
                    args = attv_q.pop(0)
                    emit_attv(*args)
                    if args[1] == N_PAIR - 1:
                        # previous stripe complete: drain its accumulator on
                        # DVE (idle at stripe start - its first exp is pair 2)
                        ot_sb = ep_pool.tile([65, 512], BF16, tag="ot_sb", bufs=2)
                        nc.vector.tensor_copy(ot_sb, stripe_ot[args[2]])
                        pending_epilogue = make_epilogue(args[2], ot_sb)
                if i == 6 and pending_epilogue is not None:
                    pending_epilogue()
                    pending_epilogue = None
                attv_q.append((pt, i, j))
        for args in attv_q:
            emit_attv(*args)
        attv_q.clear()
        last_epilogue(N_STRIPE - 1, stripe_ot[N_STRIPE - 1])


_NC_CACHE = {}


def _get_nc():
    if "nc" not in _NC_CACHE:
        _NC_CACHE["nc"] = build_kernel()
    return _NC_CACHE["nc"]


def pack_params(gamma, beta, wq, bq, wk, wv, bv, wo, bo):
    pk = np.zeros((65, PK_COLS), np.float32)
    pk[0:64, 0:64] = np.asarray(wq, np.float32)
    pk[0:64, 64:128] = np.asarray(wk, np.float32)
    pk[0:64, 128:192] = np.asarray(wv, np.float32)
    pk[0:64, 192] = np.asarray(gamma, np.float32)
    pk[0:64, 193:257] = np.asarray(wo, np.float32)
    pk[0:64, 257] = np.asarray(beta, np.float32)
    pk[64, 0:64] = np.asarray(bq, np.float32)
    pk[64, 128:192] = np.asarray(bv, np.float32)
    pk[64, 193:257] = np.asarray(bo, np.float32)
    return pk


def build_in_maps(x, gamma, beta, wq, bq, wk, wv, bv, wo, bo):
    """Per-core NEFF input dicts plus (batch, rows) scatter info per core."""
    x = np.asarray(x, dtype=np.float32)
    pk = pack_params(gamma, beta, wq, bq, wk, wv, bv, wo, bo)
    xf = x.reshape(B, S, C)
    in_maps = []
    scatter = []
    for core in range(8):
        b, h = core // 2, core % 2
        own = slice(h * SQ, (h + 1) * SQ)
        other = slice((1 - h) * SQ, (2 - h) * SQ)
        x_local = np.concatenate([xf[b][own], xf[b][other]], axis=0)
        xT = x_local.T.astype(ml_dtypes.bfloat16)          # [64, 4096]
        xT = np.ascontiguousarray(xT.reshape(64, 4, 1024).transpose(1, 0, 2))
        xq = np.ascontiguousarray(
            x_local[:SQ].reshape(SQ // 128, 128, C).transpose(1, 0, 2))
        in_maps.append({"xT": xT, "x_q": xq, "pk": pk})
        scatter.append((b, np.arange(h * SQ, (h + 1) * SQ)))
    return in_maps, scatter


def _run(in_maps, scatter, **spmd_kwargs):
    nc = _get_nc()
    res = run_bass_kernel_spmd(nc, in_maps, core_ids=list(range(8)),
                               **spmd_kwargs)
    out = np.empty((B, S, C), np.float32)
    for core in range(8):
        b, rows = scatter[core]
        r = res.results[core]["out"]  # [128, SQ//128, C] device layout
        out[b][rows] = r.transpose(1, 0, 2).reshape(SQ, C)
    return out.reshape(B, H, W, C), res


def kernel(x, gamma, beta, wq, bq, wk, bk, wv, bv, wo, bo):
    # bk is provably a no-op: it shifts each query's scores by a constant
    # which softmax cancels, so it is not shipped to the device.
    in_maps, scatter = build_in_maps(x, gamma, beta, wq, bq, wk, wv, bv, wo, bo)
    try:
        out, _ = _run(in_maps, scatter)
    except Exception:
        # transient NRT device hiccups have been observed on a first run;
        # one retry has always succeeded
        out, _ = _run(in_maps, scatter)
    return out
